# revision 13
# baseline (speedup 1.0000x reference)
"""Conv2d 3x3 via 1-D Winograd F(2,3) along the kh (row) axis.

out[2r]   = d0*g0 + d1*g1 + d2*g2   with m0=(d0-d2)g0, m1=(d1+d2)(g0+g1+g2)/2,
out[2r+1] = d1*g0 + d2*g1 + d3*g2        m2=(d2-d1)(g0-g1+g2)/2, m3=(d1-d3)g2
out[2r] = m0+m1+m2, out[2r+1] = m1-m2-m3  -> 4 multiplies per 2x1 outputs
instead of 6: a 1.5x TensorEngine FLOP cut vs direct conv.

Per core (4 images): input rows pre-transformed on DVE into D_k
(k=0..3, pairs r=0..27), weights host-transformed into G_k[ci,kw,co].
For each (image, co_tile, chunk of 8 pairs): 4 PSUM banks M_k, each
accumulating 6 matmuls (2 ci-tiles x 3 kw taps, K=128, N=pairs*56).
Output transform + bias on ACT (Identity+bias) and DVE, interleaved
rows staged in SBUF, one contiguous DMA out per chunk.
"""

import numpy as np
import ml_dtypes

import concourse.bass as bass
import concourse.mybir as mybir
from concourse import bacc
from concourse.tile import TileContext
from concourse.bass_utils import run_bass_kernel_spmd

P = 128
N_CORES = 8
NIMG = 4
CIN = 256
COUT = 256
H = W = 56
HP = WP = 58
CI_T = 2
CO_T = 2
NPAIR = 28                 # output row pairs
TOP_ROWS = 34              # padded rows 0..33  -> pairs 0..15
BOT_FIRST = 32             # padded rows 32..57 -> pairs 16..27
BOT_ROWS = HP - BOT_FIRST  # 26
TOP_PAIRS = 16
BOT_PAIRS = 12
# chunks of output pairs: (r0, npairs); 6-pair tails keep N large enough
# that LDWEIGHTS (97ns) hides under the matmul stream
CHUNKS = [(0, 8), (8, 8), (16, 6), (22, 6)]

_cached = {}


def _build_nc():
    nc = bacc.Bacc("TRN2", target_bir_lowering=False, debug=False,
                   num_devices=N_CORES)

    ip_h = nc.declare_dram_parameter("ip", [NIMG, CIN, HP, WP],
                                     mybir.dt.bfloat16, isOutput=False)
    w_h = nc.declare_dram_parameter("weight", [P, CI_T * 4 * 3 * COUT],
                                    mybir.dt.bfloat16, isOutput=False)
    b_h = nc.declare_dram_parameter("bias", [P, CO_T],
                                    mybir.dt.float32, isOutput=False)
    out_h = nc.declare_dram_parameter("out", [NIMG, COUT, H, W],
                                      mybir.dt.float32, isOutput=True)

    ip_v = ip_h.ap().rearrange("n (t p) h w -> n t p h w", p=P)
    w_v = w_h.ap()
    out_v = out_h.ap().rearrange("n (t p) h w -> n t p (h w)", p=P)

    def woff(it, k, kw):
        return ((it * 4 + k) * 3 + kw) * COUT

    with TileContext(nc) as tc:
        with (
            tc.tile_pool(name="const", bufs=1) as cpool,
            tc.tile_pool(name="pad", bufs=2) as ppool,
            tc.tile_pool(name="padf", bufs=4) as pfpool,
            tc.tile_pool(name="dt", bufs=8) as dtpool,
            tc.tile_pool(name="df", bufs=12) as dfpool,
            tc.tile_pool(name="db", bufs=8) as dbpool,
            tc.tile_pool(name="tmp", bufs=8) as tpool,
            tc.tile_pool(name="outs", bufs=4) as opool,
            tc.tile_pool(name="psum", bufs=8, space="PSUM") as pspool,
        ):
            wt = cpool.tile([P, CI_T * 4 * 3 * COUT], mybir.dt.bfloat16)
            bt = cpool.tile([P, CO_T], mybir.dt.float32)

            tops = [[None] * CI_T for _ in range(NIMG)]
            bots = [[None] * CI_T for _ in range(NIMG)]

            def _load_top(n, t):
                pt = ppool.tile([P, TOP_ROWS, WP], mybir.dt.bfloat16,
                                tag="padtop")
                nc.sync.dma_start(out=pt[:], in_=ip_v[n, t, :, 0:TOP_ROWS])
                tops[n][t] = pt

            def _load_bot(n, t):
                pb = ppool.tile([P, BOT_ROWS, WP], mybir.dt.bfloat16,
                                tag="padbot")
                nc.sync.dma_start(out=pb[:], in_=ip_v[n, t, :, BOT_FIRST:HP])
                bots[n][t] = pb

            # staged in PE-consumption order (single HWDGE queue is FIFO)
            HW0 = 4 * 3 * COUT  # per-ci-tile weight width
            nc.sync.dma_start(out=wt[:, 0:3 * COUT],
                              in_=w_v[:, 0:3 * COUT])           # it0 k0
            _load_top(0, 0)
            _load_top(0, 1)
            nc.sync.dma_start(out=wt[:, 3 * COUT:HW0],
                              in_=w_v[:, 3 * COUT:HW0])         # it0 k1-3
            nc.sync.dma_start(out=wt[:, HW0:], in_=w_v[:, HW0:])  # it1
            nc.sync.dma_start(out=bt[:], in_=b_h.ap())
            _load_bot(0, 0)
            _load_bot(0, 1)
            # images 1-3: single full pad tile (fewer DMAs + D ops)
            fulls = [[None] * CI_T for _ in range(NIMG)]
            for n in range(1, NIMG):
                for t in range(CI_T):
                    pf = pfpool.tile([P, HP, WP], mybir.dt.bfloat16,
                                     tag="padfull", name=f"pf_{n}_{t}")
                    nc.sync.dma_start(out=pf[:], in_=ip_v[n, t])
                    fulls[n][t] = pf

            # D transform: Dk_top [128,16,58], Dk_bot [128,12,58] per (n,it)
            # for image 0; merged Dk [128,28,58] for images 1-3
            dtops = [[None] * CI_T for _ in range(NIMG)]
            dbots = [[None] * CI_T for _ in range(NIMG)]
            dfull = [[None] * CI_T for _ in range(NIMG)]

            def _transform_full(n, t):
                pf = fulls[n][t]
                pv = pf.rearrange("p (r two) c -> p two r c", two=2)
                pe, po = pv[:, 0], pv[:, 1]   # 29 rows each
                ds = []
                for k in range(4):
                    df = dfpool.tile([P, NPAIR, WP], mybir.dt.bfloat16,
                                     tag="dfull", name=f"dfull_{n}_{t}_{k}")
                    ds.append(df)
                nc.vector.tensor_tensor(ds[0][:], pe[:, 0:28], pe[:, 1:29],
                                        mybir.AluOpType.subtract)
                nc.vector.tensor_tensor(ds[1][:], po[:, 0:28], pe[:, 1:29],
                                        mybir.AluOpType.add)
                nc.vector.tensor_tensor(ds[2][:], pe[:, 1:29], po[:, 0:28],
                                        mybir.AluOpType.subtract)
                nc.vector.tensor_tensor(ds[3][:], po[:, 0:28], po[:, 1:29],
                                        mybir.AluOpType.subtract)
                dfull[n][t] = ds

            def _transform(n, t):
                pt = tops[n][t]
                pev = pt.rearrange("p (r two) c -> p two r c", two=2)
                pe, po = pev[:, 0], pev[:, 1]   # even/odd local rows, 17 each
                dts = []
                for k in range(4):
                    dt = dtpool.tile([P, TOP_PAIRS, WP], mybir.dt.bfloat16,
                                     tag="dtop", name=f"dtop_{n}_{t}_{k}")
                    dts.append(dt)
                nc.vector.tensor_tensor(dts[0][:], pe[:, 0:16], pe[:, 1:17],
                                        mybir.AluOpType.subtract)
                nc.vector.tensor_tensor(dts[1][:], po[:, 0:16], pe[:, 1:17],
                                        mybir.AluOpType.add)
                nc.vector.tensor_tensor(dts[2][:], pe[:, 1:17], po[:, 0:16],
                                        mybir.AluOpType.subtract)
                nc.vector.tensor_tensor(dts[3][:], po[:, 0:16], po[:, 1:17],
                                        mybir.AluOpType.subtract)
                dtops[n][t] = dts

                pb = bots[n][t]
                pbv = pb.rearrange("p (r two) c -> p two r c", two=2)
                pe_b, po_b = pbv[:, 0], pbv[:, 1]   # 13 each
                dbs = []
                for k in range(4):
                    db = dbpool.tile([P, BOT_PAIRS, WP], mybir.dt.bfloat16,
                                     tag="dbot", name=f"dbot_{n}_{t}_{k}")
                    dbs.append(db)
                nc.vector.tensor_tensor(dbs[0][:], pe_b[:, 0:12], pe_b[:, 1:13],
                                        mybir.AluOpType.subtract)
                nc.vector.tensor_tensor(dbs[1][:], po_b[:, 0:12], pe_b[:, 1:13],
                                        mybir.AluOpType.add)
                nc.vector.tensor_tensor(dbs[2][:], pe_b[:, 1:13], po_b[:, 0:12],
                                        mybir.AluOpType.subtract)
                nc.vector.tensor_tensor(dbs[3][:], po_b[:, 0:12], po_b[:, 1:13],
                                        mybir.AluOpType.subtract)
                dbots[n][t] = dbs

            for n in range(NIMG):
                if n == 0:
                    _transform(n, 0)
                    _transform(n, 1)
                else:
                    _transform_full(n, 0)
                    _transform_full(n, 1)

                for ot in range(CO_T):
                    for (r0, pr) in CHUNKS:
                        N = pr * W
                        ms = [pspool.tile([P, N], mybir.dt.float32,
                                          name=f"m_{n}_{ot}_{r0}_{k}",
                                          tag="mpsum")
                              for k in range(4)]
                        if n == 0:
                            use_top = r0 < TOP_PAIRS
                            lr0 = r0 if use_top else r0 - TOP_PAIRS
                            dset = dtops[n] if use_top else dbots[n]
                        else:
                            lr0 = r0
                            dset = dfull[n]
                        for it in range(CI_T):
                            for k in range(4):
                                dv = dset[it][k]
                                for kw in range(3):
                                    rhs = dv[:, lr0:lr0 + pr, kw:kw + W]
                                    o = woff(it, k, kw) + ot * P
                                    nc.tensor.matmul(
                                        ms[k][:], wt[:, o:o + P], rhs,
                                        start=(it == 0 and kw == 0),
                                        stop=(it == CI_T - 1 and kw == 2),
                                    )
                        # out[2r] = M0+M1+M2+b ; out[2r+1] = M1-M2-M3+b
                        ob = opool.tile([P, pr, 2, W], mybir.dt.float32)
                        bias = bt[:, ot:ot + 1]
                        tb = tpool.tile([P, N], mybir.dt.float32, tag="ev")
                        nc.scalar.activation(
                            tb[:], ms[0][:],
                            mybir.ActivationFunctionType.Identity, bias=bias)
                        t0 = tpool.tile([P, N], mybir.dt.float32, tag="ev")
                        nc.vector.tensor_tensor(t0[:], tb[:], ms[1][:],
                                                mybir.AluOpType.add)
                        o0 = ob[:, :, 0, :]
                        nc.vector.tensor_tensor(o0, t0[:], ms[2][:],
                                                mybir.AluOpType.add)
                        ua = tpool.tile([P, N], mybir.dt.float32, tag="ev")
                        nc.scalar.activation(
                            ua[:], ms[1][:],
                            mybir.ActivationFunctionType.Identity, bias=bias)
                        u1 = tpool.tile([P, N], mybir.dt.float32, tag="ev")
                        nc.vector.tensor_tensor(u1[:], ua[:], ms[2][:],
                                                mybir.AluOpType.subtract)
                        o1 = ob[:, :, 1, :]
                        nc.vector.tensor_tensor(o1, u1[:], ms[3][:],
                                                mybir.AluOpType.subtract)
                        nc.sync.dma_start(
                            out=out_v[n, ot, :,
                                      2 * r0 * W:(2 * r0 + 2 * pr) * W],
                            in_=ob[:])
    nc.finalize()
    return nc


def _prep_inputs(ip, weight, bias):
    bf16 = ml_dtypes.bfloat16
    ipp = np.zeros((ip.shape[0], CIN, HP, WP), dtype=bf16)
    ipp[:, :, 1:57, 1:57] = ip.astype(bf16)
    # Winograd weight transform along kh: G_k[ci, kw, co]
    w0 = weight[:, :, 0, :]    # (co, ci, kw)
    w1 = weight[:, :, 1, :]
    w2 = weight[:, :, 2, :]
    g = np.stack([w0, (w0 + w1 + w2) * 0.5, (w0 - w1 + w2) * 0.5, w2],
                 axis=0)                       # (k, co, ci, kw)
    g = g.transpose(2, 0, 3, 1)                # (ci, k, kw, co)
    g = (g.reshape(CI_T, P, 4, 3, COUT)
          .transpose(1, 0, 2, 3, 4)            # (ci_p, ci_t, k, kw, co)
          .reshape(P, CI_T * 4 * 3 * COUT))
    wT = np.ascontiguousarray(g).astype(bf16)
    bT = np.ascontiguousarray(np.asarray(bias, np.float32).reshape(CO_T, P).T)
    return ipp, wT, bT


def kernel(ip, weight, bias, _trace=False, _trace_kwargs=None):
    ip = np.asarray(ip, dtype=np.float32)
    weight = np.asarray(weight, dtype=np.float32)
    bias = np.asarray(bias, dtype=np.float32)

    if "nc" not in _cached:
        _cached["nc"] = _build_nc()
    nc = _cached["nc"]

    ipp, wT, bT = _prep_inputs(ip, weight, bias)
    in_maps = [
        {"ip": ipp[i * NIMG:(i + 1) * NIMG], "weight": wT, "bias": bT}
        for i in range(N_CORES)
    ]
    res = run_bass_kernel_spmd(
        nc, in_maps, core_ids=list(range(N_CORES)),
        trace=_trace, **(_trace_kwargs or {}),
    )
    out = np.concatenate([r["out"] for r in res.results], axis=0)
    if _trace:
        return out, res
    return out


# revision 14
# speedup vs baseline: 1.1029x; 1.1029x over previous
"""Conv2d 3x3 via 1-D Winograd F(2,3) along the kh (row) axis.

out[2r]   = d0*g0 + d1*g1 + d2*g2   with m0=(d0-d2)g0, m1=(d1+d2)(g0+g1+g2)/2,
out[2r+1] = d1*g0 + d2*g1 + d3*g2        m2=(d2-d1)(g0-g1+g2)/2, m3=(d1-d3)g2
out[2r] = m0+m1+m2, out[2r+1] = m1-m2-m3  -> 4 multiplies per 2x1 outputs
instead of 6: a 1.5x TensorEngine FLOP cut vs direct conv.

Per core (4 images): input rows pre-transformed on DVE into D_k
(k=0..3, pairs r=0..27), weights host-transformed into G_k[ci,kw,co].
For each (image, co_tile, chunk of 8 pairs): 4 PSUM banks M_k, each
accumulating 6 matmuls (2 ci-tiles x 3 kw taps, K=128, N=pairs*56).
Output transform + bias on ACT (Identity+bias) and DVE, interleaved
rows staged in SBUF, one contiguous DMA out per chunk.
"""

import numpy as np
import ml_dtypes

import concourse.bass as bass
import concourse.mybir as mybir
from concourse import bacc
from concourse.tile import TileContext
from concourse.bass_utils import run_bass_kernel_spmd

P = 128
N_CORES = 8
NIMG = 4
CIN = 256
COUT = 256
H = W = 56
HP = WP = 58
CI_T = 2
CO_T = 2
NPAIR = 28                 # output row pairs
TOP_ROWS = 34              # padded rows 0..33  -> pairs 0..15
BOT_FIRST = 32             # padded rows 32..57 -> pairs 16..27
BOT_ROWS = HP - BOT_FIRST  # 26
TOP_PAIRS = 16
BOT_PAIRS = 12
# chunks of output pairs: (r0, npairs); 6-pair tails keep N large enough
# that LDWEIGHTS (97ns) hides under the matmul stream
CHUNKS = [(0, 8), (8, 8), (16, 6), (22, 6)]

_cached = {}


def _build_nc():
    nc = bacc.Bacc("TRN2", target_bir_lowering=False, debug=False,
                   num_devices=N_CORES)

    ip_h = nc.declare_dram_parameter("ip", [NIMG, CIN, HP, WP],
                                     mybir.dt.bfloat16, isOutput=False)
    w_h = nc.declare_dram_parameter("weight", [P, CI_T * 4 * 3 * COUT],
                                    mybir.dt.bfloat16, isOutput=False)
    b_h = nc.declare_dram_parameter("bias", [P, CO_T],
                                    mybir.dt.float32, isOutput=False)
    out_h = nc.declare_dram_parameter("out", [NIMG, COUT, H, W],
                                      mybir.dt.float32, isOutput=True)

    ip_v = ip_h.ap().rearrange("n (t p) h w -> n t p h w", p=P)
    w_v = w_h.ap()
    out_v = out_h.ap().rearrange("n (t p) h w -> n t p (h w)", p=P)

    def woff(it, k, kw):
        return ((it * 4 + k) * 3 + kw) * COUT

    with TileContext(nc) as tc:
        with (
            tc.tile_pool(name="const", bufs=1) as cpool,
            tc.tile_pool(name="pad", bufs=2) as ppool,
            tc.tile_pool(name="padf", bufs=4) as pfpool,
            tc.tile_pool(name="dt", bufs=8) as dtpool,
            tc.tile_pool(name="df", bufs=12) as dfpool,
            tc.tile_pool(name="db", bufs=8) as dbpool,
            tc.tile_pool(name="tmp", bufs=8) as tpool,
            tc.tile_pool(name="outs", bufs=4) as opool,
            tc.tile_pool(name="psum", bufs=8, space="PSUM") as pspool,
        ):
            wt = cpool.tile([P, CI_T * 4 * 3 * COUT], mybir.dt.bfloat16)
            bt = cpool.tile([P, CO_T], mybir.dt.float32)

            tops = [[None] * CI_T for _ in range(NIMG)]
            bots = [[None] * CI_T for _ in range(NIMG)]

            def _load_top(n, t):
                pt = ppool.tile([P, TOP_ROWS, WP], mybir.dt.bfloat16,
                                tag="padtop")
                nc.sync.dma_start(out=pt[:], in_=ip_v[n, t, :, 0:TOP_ROWS])
                tops[n][t] = pt

            def _load_bot(n, t):
                pb = ppool.tile([P, BOT_ROWS, WP], mybir.dt.bfloat16,
                                tag="padbot")
                nc.sync.dma_start(out=pb[:], in_=ip_v[n, t, :, BOT_FIRST:HP])
                bots[n][t] = pb

            # staged in PE-consumption order (single HWDGE queue is FIFO)
            HW0 = 4 * 3 * COUT  # per-ci-tile weight width
            nc.sync.dma_start(out=wt[:, 0:3 * COUT],
                              in_=w_v[:, 0:3 * COUT])           # it0 k0
            _load_top(0, 0)
            nc.sync.dma_start(out=wt[:, 3 * COUT:HW0],
                              in_=w_v[:, 3 * COUT:HW0])         # it0 k1-3
            _load_top(0, 1)
            nc.sync.dma_start(out=wt[:, HW0:], in_=w_v[:, HW0:])  # it1
            nc.sync.dma_start(out=bt[:], in_=b_h.ap())
            _load_bot(0, 0)
            _load_bot(0, 1)
            # images 1-3: single full pad tile (fewer DMAs + D ops)
            fulls = [[None] * CI_T for _ in range(NIMG)]
            for n in range(1, NIMG):
                for t in range(CI_T):
                    pf = pfpool.tile([P, HP, WP], mybir.dt.bfloat16,
                                     tag="padfull", name=f"pf_{n}_{t}")
                    nc.sync.dma_start(out=pf[:], in_=ip_v[n, t])
                    fulls[n][t] = pf

            # D transform: Dk_top [128,16,58], Dk_bot [128,12,58] per (n,it)
            # for image 0; merged Dk [128,28,58] for images 1-3
            dtops = [[None] * CI_T for _ in range(NIMG)]
            dbots = [[None] * CI_T for _ in range(NIMG)]
            dfull = [[None] * CI_T for _ in range(NIMG)]

            def _transform_full(n, t):
                pf = fulls[n][t]
                pv = pf.rearrange("p (r two) c -> p two r c", two=2)
                pe, po = pv[:, 0], pv[:, 1]   # 29 rows each
                ds = []
                for k in range(4):
                    df = dfpool.tile([P, NPAIR, WP], mybir.dt.bfloat16,
                                     tag="dfull", name=f"dfull_{n}_{t}_{k}")
                    ds.append(df)
                nc.vector.tensor_tensor(ds[0][:], pe[:, 0:28], pe[:, 1:29],
                                        mybir.AluOpType.subtract)
                nc.vector.tensor_tensor(ds[1][:], po[:, 0:28], pe[:, 1:29],
                                        mybir.AluOpType.add)
                nc.vector.tensor_tensor(ds[2][:], pe[:, 1:29], po[:, 0:28],
                                        mybir.AluOpType.subtract)
                nc.vector.tensor_tensor(ds[3][:], po[:, 0:28], po[:, 1:29],
                                        mybir.AluOpType.subtract)
                dfull[n][t] = ds

            def _transform(n, t):
                pt = tops[n][t]
                pev = pt.rearrange("p (r two) c -> p two r c", two=2)
                pe, po = pev[:, 0], pev[:, 1]   # even/odd local rows, 17 each
                dts = []
                for k in range(4):
                    dt = dtpool.tile([P, TOP_PAIRS, WP], mybir.dt.bfloat16,
                                     tag="dtop", name=f"dtop_{n}_{t}_{k}")
                    dts.append(dt)
                nc.vector.tensor_tensor(dts[0][:], pe[:, 0:16], pe[:, 1:17],
                                        mybir.AluOpType.subtract)
                nc.vector.tensor_tensor(dts[1][:], po[:, 0:16], pe[:, 1:17],
                                        mybir.AluOpType.add)
                nc.vector.tensor_tensor(dts[2][:], pe[:, 1:17], po[:, 0:16],
                                        mybir.AluOpType.subtract)
                nc.vector.tensor_tensor(dts[3][:], po[:, 0:16], po[:, 1:17],
                                        mybir.AluOpType.subtract)
                dtops[n][t] = dts

                pb = bots[n][t]
                pbv = pb.rearrange("p (r two) c -> p two r c", two=2)
                pe_b, po_b = pbv[:, 0], pbv[:, 1]   # 13 each
                dbs = []
                for k in range(4):
                    db = dbpool.tile([P, BOT_PAIRS, WP], mybir.dt.bfloat16,
                                     tag="dbot", name=f"dbot_{n}_{t}_{k}")
                    dbs.append(db)
                nc.vector.tensor_tensor(dbs[0][:], pe_b[:, 0:12], pe_b[:, 1:13],
                                        mybir.AluOpType.subtract)
                nc.vector.tensor_tensor(dbs[1][:], po_b[:, 0:12], pe_b[:, 1:13],
                                        mybir.AluOpType.add)
                nc.vector.tensor_tensor(dbs[2][:], pe_b[:, 1:13], po_b[:, 0:12],
                                        mybir.AluOpType.subtract)
                nc.vector.tensor_tensor(dbs[3][:], po_b[:, 0:12], po_b[:, 1:13],
                                        mybir.AluOpType.subtract)
                dbots[n][t] = dbs

            for n in range(NIMG):
                if n == 0:
                    _transform(n, 0)
                    _transform(n, 1)
                else:
                    _transform_full(n, 0)
                    _transform_full(n, 1)

                for ot in range(CO_T):
                    for (r0, pr) in CHUNKS:
                        N = pr * W
                        ms = [pspool.tile([P, N], mybir.dt.float32,
                                          name=f"m_{n}_{ot}_{r0}_{k}",
                                          tag="mpsum")
                              for k in range(4)]
                        if n == 0:
                            use_top = r0 < TOP_PAIRS
                            lr0 = r0 if use_top else r0 - TOP_PAIRS
                            dset = dtops[n] if use_top else dbots[n]
                        else:
                            lr0 = r0
                            dset = dfull[n]
                        for it in range(CI_T):
                            for k in range(4):
                                dv = dset[it][k]
                                for kw in range(3):
                                    rhs = dv[:, lr0:lr0 + pr, kw:kw + W]
                                    o = woff(it, k, kw) + ot * P
                                    nc.tensor.matmul(
                                        ms[k][:], wt[:, o:o + P], rhs,
                                        start=(it == 0 and kw == 0),
                                        stop=(it == CI_T - 1 and kw == 2),
                                    )
                        # out[2r] = M0+M1+M2+b ; out[2r+1] = M1-M2-M3+b
                        ob = opool.tile([P, pr, 2, W], mybir.dt.float32)
                        bias = bt[:, ot:ot + 1]
                        tb = tpool.tile([P, N], mybir.dt.float32, tag="ev")
                        nc.scalar.activation(
                            tb[:], ms[0][:],
                            mybir.ActivationFunctionType.Identity, bias=bias)
                        t0 = tpool.tile([P, N], mybir.dt.float32, tag="ev")
                        nc.vector.tensor_tensor(t0[:], tb[:], ms[1][:],
                                                mybir.AluOpType.add)
                        o0 = ob[:, :, 0, :]
                        nc.vector.tensor_tensor(o0, t0[:], ms[2][:],
                                                mybir.AluOpType.add)
                        ua = tpool.tile([P, N], mybir.dt.float32, tag="ev")
                        nc.scalar.activation(
                            ua[:], ms[1][:],
                            mybir.ActivationFunctionType.Identity, bias=bias)
                        u1 = tpool.tile([P, N], mybir.dt.float32, tag="ev")
                        nc.vector.tensor_tensor(u1[:], ua[:], ms[2][:],
                                                mybir.AluOpType.subtract)
                        o1 = ob[:, :, 1, :]
                        nc.vector.tensor_tensor(o1, u1[:], ms[3][:],
                                                mybir.AluOpType.subtract)
                        nc.sync.dma_start(
                            out=out_v[n, ot, :,
                                      2 * r0 * W:(2 * r0 + 2 * pr) * W],
                            in_=ob[:])
    nc.finalize()
    return nc


def _prep_inputs(ip, weight, bias):
    bf16 = ml_dtypes.bfloat16
    ipp = np.zeros((ip.shape[0], CIN, HP, WP), dtype=bf16)
    ipp[:, :, 1:57, 1:57] = ip.astype(bf16)
    # Winograd weight transform along kh: G_k[ci, kw, co]
    w0 = weight[:, :, 0, :]    # (co, ci, kw)
    w1 = weight[:, :, 1, :]
    w2 = weight[:, :, 2, :]
    g = np.stack([w0, (w0 + w1 + w2) * 0.5, (w0 - w1 + w2) * 0.5, w2],
                 axis=0)                       # (k, co, ci, kw)
    g = g.transpose(2, 0, 3, 1)                # (ci, k, kw, co)
    g = (g.reshape(CI_T, P, 4, 3, COUT)
          .transpose(1, 0, 2, 3, 4)            # (ci_p, ci_t, k, kw, co)
          .reshape(P, CI_T * 4 * 3 * COUT))
    wT = np.ascontiguousarray(g).astype(bf16)
    bT = np.ascontiguousarray(np.asarray(bias, np.float32).reshape(CO_T, P).T)
    return ipp, wT, bT


def kernel(ip, weight, bias, _trace=False, _trace_kwargs=None):
    ip = np.asarray(ip, dtype=np.float32)
    weight = np.asarray(weight, dtype=np.float32)
    bias = np.asarray(bias, dtype=np.float32)

    if "nc" not in _cached:
        _cached["nc"] = _build_nc()
    nc = _cached["nc"]

    ipp, wT, bT = _prep_inputs(ip, weight, bias)
    in_maps = [
        {"ip": ipp[i * NIMG:(i + 1) * NIMG], "weight": wT, "bias": bT}
        for i in range(N_CORES)
    ]
    res = run_bass_kernel_spmd(
        nc, in_maps, core_ids=list(range(N_CORES)),
        trace=_trace, **(_trace_kwargs or {}),
    )
    out = np.concatenate([r["out"] for r in res.results], axis=0)
    if _trace:
        return out, res
    return out
